# revision 4
# baseline (speedup 1.0000x reference)
"""Trainium2 Bass kernel for nn_EquiConv (e3nn-style tensor product with
per-edge generated weights), data-parallel over edges on 8 NeuronCores.

Per edge e:
  h = silu(fw @ W1n)                    [64]   (mm1, PE, fp32r)
  w = h @ W2n                           [2304] (mm2, PE, fp32r; W2n pre-scaled
                                                by SILU_NORM/sqrt(64))
  out0[w] = sum_u a0[u]   * w[u*32+w]          (u in 0..47)
  c[w]    = sum_u x1_0[u] * w[1536+u*16+w]
  d[k,w]  = sum_u x1s[u,k]* w[2048+u*16+w]
  out1[w,k] = x2c[k]*c[w] + d[k,w]
with a0 = [pw00*x2_0*x1_0, pw110/sqrt3 * (x1_1 . x2_1)],
     x1s = pw101/sqrt3 * x2_0 * x1_1, x2c = pw011/sqrt3 * x2_1.

Contraction runs on DVE: broadcast-multiply (step-0 AP) + strided reduce,
with edges on partitions (128-edge blocks).
"""
import math

import numpy as np

E_TOTAL = 65536
N_CORES = 8
E_CORE = E_TOTAL // N_CORES        # 8192
TILE_E = 512                       # edges per tile
BLK = 128                          # edges per partition block
N_TILES = E_CORE // TILE_E         # 16
MUL0, MUL1 = 32, 16
FC_IN, FC_HID = 64, 64
WNUMEL = 2304
SILU_NORM = 1.6790
ISQRT3 = 1.0 / math.sqrt(3.0)
PW00 = math.sqrt(1.0 / (MUL0 * 2))            # 0.125
PW110I3 = math.sqrt(1.0 / (MUL1 * 2)) * ISQRT3
PW011I3 = math.sqrt(3.0 / (MUL0 * 2)) * ISQRT3
PW101I3 = math.sqrt(3.0 / (MUL1 * 2)) * ISQRT3

_NC_CACHE = {}


def _build():
    import concourse.tile as tile
    from concourse import bacc, mybir
    from concourse.masks import make_identity

    f32 = mybir.dt.float32
    f32r = mybir.dt.float32r
    MULT = mybir.AluOpType.mult
    ADD = mybir.AluOpType.add
    AXX = mybir.AxisListType.X

    nc = bacc.Bacc("TRN2", target_bir_lowering=False, debug=False)
    fea_in1 = nc.declare_dram_parameter("fea_in1", [E_CORE, 80], f32, isOutput=False)
    fea_in2 = nc.declare_dram_parameter("fea_in2", [E_CORE, 4], f32, isOutput=False)
    fea_w = nc.declare_dram_parameter("fea_weight", [E_CORE, 64], f32, isOutput=False)
    W1n = nc.declare_dram_parameter("W1n", [64, 64], f32, isOutput=False)
    W2n = nc.declare_dram_parameter("W2n", [64, WNUMEL], f32, isOutput=False)
    out_d = nc.declare_dram_parameter("out", [E_CORE, 80], f32, isOutput=True)

    with tile.TileContext(nc) as tc:
        with (
            tc.tile_pool(name="consts", bufs=1) as consts,
            tc.tile_pool(name="ins", bufs=3) as insp,
            tc.tile_pool(name="mid", bufs=3) as mid,
            tc.tile_pool(name="work", bufs=2) as work,
            tc.tile_pool(name="outs", bufs=3) as outsp,
            tc.tile_pool(name="ps_w", bufs=1, space="PSUM") as ps_w,
            tc.tile_pool(name="ps_s", bufs=1, space="PSUM") as ps_s,
        ):
            ident = consts.tile([128, 128], f32)
            make_identity(nc, ident)
            w1_t = consts.tile([64, 64], f32r)
            nc.gpsimd.dma_start(w1_t[:], W1n[:])
            w2_t = consts.tile([64, WNUMEL], f32r)
            nc.gpsimd.dma_start(w2_t[:], W2n[:])

            for t in range(N_TILES):
                e0 = t * TILE_E
                # ---- load fw slab, transpose to [64, 512] ----
                fw_blks = []
                for b in range(4):
                    fwb = insp.tile([BLK, 64], f32, tag="fwb")
                    nc.sync.dma_start(
                        fwb[:], fea_w[e0 + b * BLK:e0 + (b + 1) * BLK, :])
                    fw_blks.append(fwb)
                fwT_ps = ps_s.tile([64, TILE_E], f32, tag="fwT")
                for b in range(4):
                    nc.tensor.transpose(
                        fwT_ps[:, b * BLK:(b + 1) * BLK], fw_blks[b][:], ident[:])
                fwT_sb = mid.tile([64, TILE_E], f32r, tag="fwT_sb")
                nc.scalar.copy(fwT_sb[:], fwT_ps[:])

                # ---- mm1 + silu -> h [64, 512] (f32r) ----
                h_ps = ps_s.tile([64, TILE_E], f32, tag="h")
                nc.tensor.matmul(h_ps[:], w1_t[:], fwT_sb[:], start=True, stop=True)
                h_sb = mid.tile([64, TILE_E], f32r, tag="h_sb")
                nc.scalar.activation(
                    h_sb[:], h_ps[:], mybir.ActivationFunctionType.Silu)

                for b in range(4):
                    eb = e0 + b * BLK
                    # ---- mm2: w [128e, 2304] in PSUM ----
                    w_ps = ps_w.tile([BLK, WNUMEL], f32, tag="w")
                    lhs = h_sb[:, b * BLK:(b + 1) * BLK]
                    for s in range(4):
                        nc.tensor.matmul(
                            w_ps[:, s * 512:(s + 1) * 512], lhs,
                            w2_t[:, s * 512:(s + 1) * 512], start=True, stop=True)
                    nc.tensor.matmul(
                        w_ps[:, 2048:2304], lhs, w2_t[:, 2048:2304],
                        start=True, stop=True)

                    # ---- per-edge features ----
                    x1b = insp.tile([BLK, 80], f32, tag="x1b")
                    nc.sync.dma_start(x1b[:], fea_in1[eb:eb + BLK, :])
                    x2b = insp.tile([BLK, 4], f32, tag="x2b")
                    nc.sync.dma_start(x2b[:], fea_in2[eb:eb + BLK, :])

                    a0 = work.tile([BLK, 48], f32, tag="a0")
                    # a0[:, :32] = pw00 * x2_0 * x1_0
                    nc.vector.tensor_scalar(
                        a0[:, 0:32], x1b[:, 0:32], x2b[:, 0:1], PW00, MULT, MULT)
                    # b[u] = sum_i x1_1[u,i]*x2_1[i] ; a0[:,32:48] = PW110I3*b
                    bvec = work.tile([BLK, 16], f32, tag="bvec")
                    x11 = x1b[:, 32:80]
                    nc.vector.tensor_scalar_mul(
                        bvec[:], x11.rearrange("p (u i) -> p i u", i=3)[:, 0],
                        x2b[:, 1:2])
                    for i in (1, 2):
                        nc.vector.scalar_tensor_tensor(
                            bvec[:], x11.rearrange("p (u i) -> p i u", i=3)[:, i],
                            x2b[:, 1 + i:2 + i], bvec[:], MULT, ADD)
                    nc.vector.tensor_scalar_mul(a0[:, 32:48], bvec[:], PW110I3)
                    # x1s = PW101I3 * x2_0 * x1_1   [128, 48]
                    x1s = work.tile([BLK, 48], f32, tag="x1s")
                    nc.vector.tensor_scalar(
                        x1s[:], x11, x2b[:, 0:1], PW101I3, MULT, MULT)
                    # x2c = PW011I3 * x2_1          [128, 3]
                    x2c = work.tile([BLK, 3], f32, tag="x2c")
                    nc.vector.tensor_scalar_mul(x2c[:], x2b[:, 1:4], PW011I3)

                    # ---- contraction ----
                    outblk = outsp.tile([BLK, 80], f32, tag="outblk")
                    # path00+110: out0[w] = sum_u a0[u]*w[(u,w)]
                    tmp00 = work.tile([BLK, 1536], f32, tag="tmp00")
                    nc.vector.tensor_tensor(
                        tmp00[:].rearrange("p (u w) -> p u w", u=48),
                        w_ps[:, 0:1536].rearrange("p (u w) -> p u w", u=48),
                        a0[:].unsqueeze(2).broadcast_to((BLK, 48, 32)),
                        MULT)
                    nc.vector.tensor_reduce(
                        outblk[:, 0:32],
                        tmp00[:].rearrange("p (u w) -> p w u", u=48), AXX, ADD)
                    # path011: c[w] = sum_u x1_0[u]*w011[(u,w)]
                    tmp011 = work.tile([BLK, 512], f32, tag="tmp011")
                    nc.vector.tensor_tensor(
                        tmp011[:].rearrange("p (u w) -> p u w", u=32),
                        w_ps[:, 1536:2048].rearrange("p (u w) -> p u w", u=32),
                        x1b[:, 0:32].unsqueeze(2).broadcast_to((BLK, 32, 16)),
                        MULT)
                    cvec = work.tile([BLK, 16], f32, tag="cvec")
                    nc.vector.tensor_reduce(
                        cvec[:], tmp011[:].rearrange("p (u w) -> p w u", u=32),
                        AXX, ADD)
                    # path101: d[k,w] = sum_u x1s[u,k]*w101[(u,w)]
                    dd = work.tile([BLK, 48], f32, tag="dd")
                    tmp101 = work.tile([BLK, 256], f32, tag="tmp101")
                    for k in range(3):
                        x1sk = x1s[:].rearrange("p (u i) -> p i u", i=3)[:, k]
                        nc.vector.tensor_tensor(
                            tmp101[:].rearrange("p (u w) -> p u w", u=16),
                            w_ps[:, 2048:2304].rearrange("p (u w) -> p u w", u=16),
                            x1sk.unsqueeze(2).broadcast_to((BLK, 16, 16)),
                            MULT)
                        nc.vector.tensor_reduce(
                            dd[:, k * 16:(k + 1) * 16],
                            tmp101[:].rearrange("p (u w) -> p w u", u=16),
                            AXX, ADD)
                    # out1[w,k] = x2c[k]*c[w] + d[k,w]  (dd k-major)
                    for k in range(3):
                        nc.vector.scalar_tensor_tensor(
                            dd[:, k * 16:(k + 1) * 16], cvec[:],
                            x2c[:, k:k + 1], dd[:, k * 16:(k + 1) * 16],
                            MULT, ADD)
                    # permute (k,w) -> (w,k) into outblk[:, 32:80]
                    nc.vector.tensor_copy(
                        outblk[:, 32:80].rearrange("p (w k) -> p w k", k=3),
                        dd[:].rearrange("p (k w) -> p w k", k=3))
                    nc.sync.dma_start(out_d[eb:eb + BLK, :], outblk[:])

    nc.finalize()
    return nc


def kernel(fea_in1, fea_in2, fea_weight, W1, W2):
    from concourse.bass_utils import run_bass_kernel_spmd

    if "nc" not in _NC_CACHE:
        _NC_CACHE["nc"] = _build()
    nc = _NC_CACHE["nc"]

    W1n = (W1 / math.sqrt(FC_IN)).astype(np.float32)
    W2n = (W2 * (SILU_NORM / math.sqrt(FC_HID))).astype(np.float32)
    fea_in1 = np.ascontiguousarray(fea_in1, dtype=np.float32)
    fea_in2 = np.ascontiguousarray(fea_in2, dtype=np.float32)
    fea_weight = np.ascontiguousarray(fea_weight, dtype=np.float32)

    in_maps = []
    for c in range(N_CORES):
        sl = slice(c * E_CORE, (c + 1) * E_CORE)
        in_maps.append({
            "fea_in1": fea_in1[sl],
            "fea_in2": fea_in2[sl],
            "fea_weight": fea_weight[sl],
            "W1n": W1n,
            "W2n": W2n,
        })
    res = run_bass_kernel_spmd(nc, in_maps, list(range(N_CORES)))
    return np.concatenate([res.results[c]["out"] for c in range(N_CORES)], axis=0)


# revision 5
# speedup vs baseline: 1.6298x; 1.6298x over previous
"""Trainium2 Bass kernel for nn_EquiConv (e3nn-style tensor product with
per-edge generated weights), data-parallel over edges on 8 NeuronCores.

Pipeline per 128-edge block (edges on partitions):
  PE : fwT = transpose(fw); hT = W1n^T @ fwT (fp32r); w = hT-slice^T @ W2n' (fp32r)
  ACT: h = silu(hT); evacuate w PSUM -> SBUF bf16
  DVE: broadcast-mul + contiguous reduce per path (bf16 2x mode), all per-edge
       scalars via step-0 broadcast APs
W2n' is host-side permuted to w-major column order per path so every DVE
access is unit-stride:
  path00/110: col w*48+u  <- w00[u,w] (u<32) | w110[u-32,w]
  path011   : 1536 + w*32 + u
  path101   : 2048 + w*16 + u
"""
import math

import numpy as np

E_TOTAL = 65536
N_CORES = 8
E_CORE = E_TOTAL // N_CORES        # 8192
TILE_E = 512
BLK = 128
N_TILES = E_CORE // TILE_E         # 16
MUL0, MUL1 = 32, 16
FC_IN, FC_HID = 64, 64
WNUMEL = 2304
SILU_NORM = 1.6790
ISQRT3 = 1.0 / math.sqrt(3.0)
PW00 = math.sqrt(1.0 / (MUL0 * 2))
PW110I3 = math.sqrt(1.0 / (MUL1 * 2)) * ISQRT3
PW011I3 = math.sqrt(3.0 / (MUL0 * 2)) * ISQRT3
PW101I3 = math.sqrt(3.0 / (MUL1 * 2)) * ISQRT3

_NC_CACHE = {}


def _w2_perm():
    """old column index for each new (w-major) column."""
    old = np.empty(WNUMEL, np.int64)
    for w in range(32):
        for u in range(48):
            old[w * 48 + u] = (u * 32 + w) if u < 32 else (1024 + (u - 32) * 32 + w)
    for w in range(16):
        for u in range(32):
            old[1536 + w * 32 + u] = 1536 + u * 16 + w
    for w in range(16):
        for u in range(16):
            old[2048 + w * 16 + u] = 2048 + u * 16 + w
    return old


def _build():
    import concourse.tile as tile
    from concourse import bacc, mybir
    from concourse.masks import make_identity

    f32 = mybir.dt.float32
    f32r = mybir.dt.float32r
    bf16 = mybir.dt.bfloat16
    MULT = mybir.AluOpType.mult
    ADD = mybir.AluOpType.add
    AXX = mybir.AxisListType.X

    nc = bacc.Bacc("TRN2", target_bir_lowering=False, debug=False)
    fea_in1 = nc.declare_dram_parameter("fea_in1", [E_CORE, 80], f32, isOutput=False)
    fea_in2 = nc.declare_dram_parameter("fea_in2", [E_CORE, 4], f32, isOutput=False)
    fea_w = nc.declare_dram_parameter("fea_weight", [E_CORE, 64], f32, isOutput=False)
    W1n = nc.declare_dram_parameter("W1n", [64, 64], f32, isOutput=False)
    W2n = nc.declare_dram_parameter("W2n", [64, WNUMEL], f32, isOutput=False)
    out_d = nc.declare_dram_parameter("out", [E_CORE, 80], f32, isOutput=True)

    with tile.TileContext(nc) as tc, nc.allow_low_precision("bf16 contraction"):
        with (
            tc.tile_pool(name="consts", bufs=1) as consts,
            tc.tile_pool(name="ins", bufs=3) as insp,
            tc.tile_pool(name="mid", bufs=2) as mid,
            tc.tile_pool(name="work", bufs=2) as work,
            tc.tile_pool(name="outs", bufs=3) as outsp,
            tc.tile_pool(name="ps_w", bufs=1, space="PSUM") as ps_w,
            tc.tile_pool(name="ps_s", bufs=1, space="PSUM") as ps_s,
        ):
            ident = consts.tile([128, 128], f32)
            make_identity(nc, ident)
            w1_t = consts.tile([64, 64], f32r)
            nc.gpsimd.dma_start(w1_t[:], W1n[:])
            w2_t = consts.tile([64, WNUMEL], f32r)
            nc.gpsimd.dma_start(w2_t[:], W2n[:])

            for t in range(N_TILES):
                e0 = t * TILE_E
                fw_blks = []
                for b in range(4):
                    fwb = insp.tile([BLK, 64], f32, tag="fwb")
                    nc.sync.dma_start(
                        fwb[:], fea_w[e0 + b * BLK:e0 + (b + 1) * BLK, :])
                    fw_blks.append(fwb)
                fwT_ps = ps_s.tile([64, TILE_E], f32, tag="fwT")
                for b in range(4):
                    nc.tensor.transpose(
                        fwT_ps[:, b * BLK:(b + 1) * BLK], fw_blks[b][:], ident[:])
                fwT_sb = mid.tile([64, TILE_E], f32r, tag="fwT_sb")
                nc.scalar.copy(fwT_sb[:], fwT_ps[:])

                h_ps = ps_s.tile([64, TILE_E], f32, tag="h")
                nc.tensor.matmul(h_ps[:], w1_t[:], fwT_sb[:], start=True, stop=True)
                h_sb = mid.tile([64, TILE_E], f32r, tag="h_sb")
                nc.scalar.activation(
                    h_sb[:], h_ps[:], mybir.ActivationFunctionType.Silu)

                for b in range(4):
                    eb = e0 + b * BLK
                    # ---- mm2 -> PSUM, then ACT-evacuate to SBUF bf16 ----
                    w_ps = ps_w.tile([BLK, WNUMEL], f32, tag="w")
                    lhs = h_sb[:, b * BLK:(b + 1) * BLK]
                    for s in range(4):
                        nc.tensor.matmul(
                            w_ps[:, s * 512:(s + 1) * 512], lhs,
                            w2_t[:, s * 512:(s + 1) * 512], start=True, stop=True)
                    nc.tensor.matmul(
                        w_ps[:, 2048:2304], lhs, w2_t[:, 2048:2304],
                        start=True, stop=True)
                    w_sb = work.tile([BLK, WNUMEL], bf16, tag="w_sb")
                    nc.scalar.copy(w_sb[:], w_ps[:])

                    # ---- per-edge features ----
                    x1b = insp.tile([BLK, 80], f32, tag="x1b")
                    nc.sync.dma_start(x1b[:], fea_in1[eb:eb + BLK, :])
                    x2b = insp.tile([BLK, 4], f32, tag="x2b")
                    nc.sync.dma_start(x2b[:], fea_in2[eb:eb + BLK, :])
                    x11 = x1b[:, 32:80]

                    a0 = work.tile([BLK, 48], bf16, tag="a0")
                    nc.vector.tensor_scalar(
                        a0[:, 0:32], x1b[:, 0:32], x2b[:, 0:1], PW00, MULT, MULT)
                    bvec = work.tile([BLK, 16], f32, tag="bvec")
                    nc.vector.tensor_scalar_mul(
                        bvec[:], x11.rearrange("p (u i) -> p i u", i=3)[:, 0],
                        x2b[:, 1:2])
                    for i in (1, 2):
                        nc.vector.scalar_tensor_tensor(
                            bvec[:], x11.rearrange("p (u i) -> p i u", i=3)[:, i],
                            x2b[:, 1 + i:2 + i], bvec[:], MULT, ADD)
                    nc.vector.tensor_scalar_mul(a0[:, 32:48], bvec[:], PW110I3)
                    # x1_0 in bf16 for path011 mul
                    x10 = work.tile([BLK, 32], bf16, tag="x10")
                    nc.vector.tensor_copy(x10[:], x1b[:, 0:32])
                    # x1sT[k,u] = PW101I3 * x2_0 * x1_1[u,k]  (k-major, bf16)
                    x1sT = work.tile([BLK, 48], bf16, tag="x1sT")
                    nc.vector.tensor_scalar(
                        x1sT[:].rearrange("p (k u) -> p k u", k=3),
                        x11.rearrange("p (u k) -> p k u", k=3),
                        x2b[:, 0:1], PW101I3, MULT, MULT)
                    x2c = work.tile([BLK, 3], f32, tag="x2c")
                    nc.vector.tensor_scalar_mul(x2c[:], x2b[:, 1:4], PW011I3)

                    # ---- contraction (all bf16, unit-stride) ----
                    outblk = outsp.tile([BLK, 80], bf16, tag="outblk")
                    tmp00 = work.tile([BLK, 1536], bf16, tag="tmp00")
                    nc.vector.tensor_tensor(
                        tmp00[:].rearrange("p (w u) -> p w u", w=32),
                        w_sb[:, 0:1536].rearrange("p (w u) -> p w u", w=32),
                        a0[:].unsqueeze(1).broadcast_to((BLK, 32, 48)),
                        MULT)
                    nc.vector.tensor_reduce(
                        outblk[:, 0:32],
                        tmp00[:].rearrange("p (w u) -> p w u", w=32), AXX, ADD)
                    tmp011 = work.tile([BLK, 512], bf16, tag="tmp011")
                    nc.vector.tensor_tensor(
                        tmp011[:].rearrange("p (w u) -> p w u", w=16),
                        w_sb[:, 1536:2048].rearrange("p (w u) -> p w u", w=16),
                        x10[:].unsqueeze(1).broadcast_to((BLK, 16, 32)),
                        MULT)
                    cvec = work.tile([BLK, 16], bf16, tag="cvec")
                    nc.vector.tensor_reduce(
                        cvec[:], tmp011[:].rearrange("p (w u) -> p w u", w=16),
                        AXX, ADD)
                    dd = work.tile([BLK, 48], bf16, tag="dd")
                    tmp101 = work.tile([BLK, 256], bf16, tag="tmp101")
                    for k in range(3):
                        nc.vector.tensor_tensor(
                            tmp101[:].rearrange("p (w u) -> p w u", w=16),
                            w_sb[:, 2048:2304].rearrange("p (w u) -> p w u", w=16),
                            x1sT[:, k * 16:(k + 1) * 16].unsqueeze(1)
                                .broadcast_to((BLK, 16, 16)),
                            MULT)
                        nc.vector.tensor_reduce(
                            dd[:, k * 16:(k + 1) * 16],
                            tmp101[:].rearrange("p (w u) -> p w u", w=16),
                            AXX, ADD)
                    for k in range(3):
                        nc.vector.scalar_tensor_tensor(
                            dd[:, k * 16:(k + 1) * 16], cvec[:],
                            x2c[:, k:k + 1], dd[:, k * 16:(k + 1) * 16],
                            MULT, ADD)
                    nc.vector.tensor_copy(
                        outblk[:, 32:80].rearrange("p (w k) -> p w k", k=3),
                        dd[:].rearrange("p (k w) -> p w k", k=3))
                    nc.gpsimd.dma_start(out_d[eb:eb + BLK, :], outblk[:])

    nc.finalize()
    return nc


def kernel(fea_in1, fea_in2, fea_weight, W1, W2):
    from concourse.bass_utils import run_bass_kernel_spmd

    if "nc" not in _NC_CACHE:
        _NC_CACHE["nc"] = _build()
    nc = _NC_CACHE["nc"]

    W1n = (W1 / math.sqrt(FC_IN)).astype(np.float32)
    W2n = (W2 * (SILU_NORM / math.sqrt(FC_HID))).astype(np.float32)[:, _w2_perm()]
    W2n = np.ascontiguousarray(W2n)
    fea_in1 = np.ascontiguousarray(fea_in1, dtype=np.float32)
    fea_in2 = np.ascontiguousarray(fea_in2, dtype=np.float32)
    fea_weight = np.ascontiguousarray(fea_weight, dtype=np.float32)

    in_maps = []
    for c in range(N_CORES):
        sl = slice(c * E_CORE, (c + 1) * E_CORE)
        in_maps.append({
            "fea_in1": fea_in1[sl],
            "fea_in2": fea_in2[sl],
            "fea_weight": fea_weight[sl],
            "W1n": W1n,
            "W2n": W2n,
        })
    res = run_bass_kernel_spmd(nc, in_maps, list(range(N_CORES)))
    return np.concatenate([res.results[c]["out"] for c in range(N_CORES)], axis=0)
